# revision 24
# baseline (speedup 1.0000x reference)
"""Multi-head attention (B=4, Q=K=2048, D=512, H=8) on 8 TRN2 NeuronCores.

Sharding: data-parallel over batch across core pairs (4 batches x 2 cores),
tensor-parallel over heads within each pair (each core owns 4 of the 8 heads:
column-sharded W_q/W_k/W_v, row-sharded W_o).  Each core emits a partial
output projection for its batch; the host sums the two partials per batch.

Device-side layout choices:
  * All activations live transposed ([feature, seq]) so every matmul contracts
    over the partition dim with no on-chip transposes.
  * Scores are computed transposed (S_T[k, q] = K_h @ Q_h^T) so the valid-len
    key padding mask is a per-partition bias on the ACT exp instruction, and
    softmax needs no max-subtraction pass (scores are O(1) here; exp of the
    -1e6 masked entries underflows to exactly 0, matching the reference).
  * A ones-column interleaved into V makes the attnV matmul emit the softmax
    denominator for free (output row 64 of each head's [65, q] PSUM tile).
  * The key dim is truncated to max(valid_lens) rounded up to 128: dropped
    keys all have softmax weight exactly 0, so this is exact.
  * The whole matmul pipeline runs in bf16 with fp32 PSUM accumulation
    (plain fp32 matmuls are 4x slower on the PE and fp32 weight loads can't
    use fast-weight-load); softmax/normalization stay fp32.  Host converts
    inputs to bf16, which also halves the input DMA traffic.
"""

import ml_dtypes
import numpy as np

import concourse.bacc as bacc
import concourse.bass as bass
import concourse.mybir as mybir
from concourse import tile
from concourse.bass_utils import run_bass_kernel_spmd

F32 = mybir.dt.float32
F32R = mybir.dt.float32r
BF16 = mybir.dt.bfloat16

B, Q, KSEQ, D, H = 4, 2048, 2048, 512, 8
DH = D // H          # 64  head dim
HL = H // 2          # 4   local heads per core
DL = HL * DH         # 256 local features per core
NEG = -1.0e6
N_CORES = 8


def build_nc(KT: int):
    """Build the single-core SPMD program for a key length of KT (mult of 128)."""
    assert KT % 128 == 0 and 128 <= KT <= KSEQ
    KTC = KT // 128                      # number of 128-wide key chunks
    NQ = Q // 512                        # 4 q-chunks of 512
    KCH = [(s, min(512, KT - s)) for s in range(0, KT, 512)]
    EXP = mybir.ActivationFunctionType.Exp

    nc = bacc.Bacc("TRN2", target_bir_lowering=False, debug=False,
                   num_devices=N_CORES)

    def din(name, shape, dt=BF16):
        return nc.dram_tensor(name, shape, dt, kind="ExternalInput").ap()

    xq_d = din("xq_t", [D, Q])
    xk_d = din("xk_t", [D, KT])
    xv_d = din("xv_t", [D, KT])
    wq_d = din("wq_t", [D, DL])
    wk_d = din("wk_t", [D, DL])
    wv_d = din("wv_t", [D, DL])
    wo_d = din("wo_t", [DL, D])
    mask_d = din("mask", [128, KTC], F32)
    y_d = nc.dram_tensor("y_t", [D, Q], F32, kind="ExternalOutput").ap()

    with tile.TileContext(nc) as tc:
        with (
            # bf16 rounding on PSUM->SBUF copies is deliberate (see docstring)
            nc.allow_low_precision(reason="bf16 matmul operands"),
            tc.tile_pool(name="persist", bufs=1) as pp,
            tc.tile_pool(name="xpool", bufs=8) as xp,
            tc.tile_pool(name="cbuf", bufs=1) as cb,
            # 8 PSUM banks: psA 2x[128,512] (projections / broadcast / output
            # projection), psS 2x[128,1024] score tiles, psO 2x[65,512]
            # attention accumulators.
            tc.tile_pool(name="psA", bufs=2, space=bass.MemorySpace.PSUM) as psA,
            tc.tile_pool(name="psS", bufs=2, space=bass.MemorySpace.PSUM) as psS,
            tc.tile_pool(name="psO", bufs=1, space=bass.MemorySpace.PSUM) as psO,
        ):
            # ---- constants / weights / mask ----
            wq = [pp.tile([128, DL], BF16, tag=f"wq{i}", name=f"wq{i}") for i in range(4)]
            wk = [pp.tile([128, DL], BF16, tag=f"wk{i}", name=f"wk{i}") for i in range(4)]
            wv = [pp.tile([128, DL], BF16, tag=f"wv{i}", name=f"wv{i}") for i in range(4)]
            wo = [pp.tile([64, D], BF16, tag=f"wo{i}", name=f"wo{i}") for i in range(HL)]
            for i in range(4):
                nc.sync.dma_start(wq[i][:], wq_d[i * 128:(i + 1) * 128, :])
            mask_sb = pp.tile([128, KTC], F32, tag="mask", name="mask_sb")
            nc.sync.dma_start(mask_sb[:], mask_d[:])
            onescr = pp.tile([128, DH], F32, tag="onescr", name="onescr")
            nc.vector.memset(onescr[:], 1.0)
            # row 64 is the broadcast-matmul lhsT (must share base partition
            # with the denominator row it multiplies against)
            ones_sb = pp.tile([65, DH], F32R, tag="ones", name="ones_sb")
            nc.vector.tensor_copy(ones_sb[64:65, :], onescr[64:65, :])

            # ---- Q projection:  q_t[o, q] = (Wq_loc/8) @ x_q  (transposed) ----
            # first input is DMA'd in 512-column chunks so the first matmul
            # group can start as early as possible
            xq = [xp.tile([128, Q], BF16, tag="x", name=f"x{i}") for i in range(4)]
            for qs in range(NQ):
                for i in range(4):
                    nc.sync.dma_start(
                        xq[i][:, qs * 512:(qs + 1) * 512],
                        xq_d[i * 128:(i + 1) * 128, qs * 512:(qs + 1) * 512])
            for i in range(4):
                nc.sync.dma_start(wk[i][:], wk_d[i * 128:(i + 1) * 128, :])
                nc.sync.dma_start(wv[i][:], wv_d[i * 128:(i + 1) * 128, :])
            for i in range(HL):
                nc.sync.dma_start(wo[i][:], wo_d[i * 64:(i + 1) * 64, :])
            q_t = [pp.tile([128, Q], BF16, tag=f"q_t{i}", name=f"q_t{i}") for i in range(2)]

            def qproj(ot, qs):
                ps = psA.tile([128, 512], F32, tag="proj", name="ps")
                for ic in range(4):
                    nc.tensor.matmul(
                        ps[:],
                        (wq[ic][:, ot * 128:(ot + 1) * 128]),
                        (xq[ic][:, qs * 512:(qs + 1) * 512]),
                        start=(ic == 0), stop=(ic == 3))
                nc.vector.tensor_copy(q_t[ot][:, qs * 512:(qs + 1) * 512], ps[:])

            for qs in range(NQ):
                qproj(0, qs)

            # ---- K projection:  k_t[o, k] ----
            xk = [xp.tile([128, Q], BF16, tag="x", name=f"x{i}") for i in range(4)]
            for i in range(4):
                nc.sync.dma_start(xk[i][:, :KT], xk_d[i * 128:(i + 1) * 128, :])
            k_t = [pp.tile([128, KT], BF16, tag=f"k_t{i}", name=f"k_t{i}") for i in range(2)]

            def kproj(ot, s, w):
                ps = psA.tile([128, 512], F32, tag="proj", name="ps")
                for ic in range(4):
                    nc.tensor.matmul(
                        ps[:, :w],
                        (wk[ic][:, ot * 128:(ot + 1) * 128]),
                        (xk[ic][:, s:s + w]),
                        start=(ic == 0), stop=(ic == 3))
                nc.vector.tensor_copy(k_t[ot][:, s:s + w], ps[:, :w])

            for (s, w) in KCH:
                kproj(0, s, w)
            for qs in range(NQ):
                qproj(1, qs)
            for (s, w) in KCH:
                kproj(1, s, w)

            # ---- V projection (emitted lazily, interleaved into the first
            # attention panel so the PE stream has no separate V phase and the
            # ACT engine starts on exps ~35us earlier).  Natural layout
            # v[k, o], heads interleaved with a ones column: per key-chunk
            # tile [128, 4*65], col h*65+64 == 1 (gives the softmax
            # denominator for free in the attnV matmul).
            xv = [xp.tile([128, Q], BF16, tag="x", name=f"x{i}") for i in range(4)]
            for i in range(4):
                nc.sync.dma_start(xv[i][:, :KT], xv_d[i * 128:(i + 1) * 128, :])
            v_sb = [pp.tile([128, HL * 65], BF16, tag=f"v{kt}", name=f"v{kt}") for kt in range(KTC)]

            def vproj(kt):
                ps = psA.tile([128, 512], F32, tag="proj", name="ps")
                for ic in range(4):
                    nc.tensor.matmul(
                        ps[:, :DL],
                        (xv[ic][:, kt * 128:(kt + 1) * 128]),
                        (wv[ic][:]),
                        start=(ic == 0), stop=(ic == 3))
                nc.vector.tensor_copy(v_sb[kt][:, 64::65], onescr[:, 0:HL])
                for h in range(HL):
                    nc.vector.tensor_copy(
                        v_sb[kt][:, h * 65:h * 65 + 64],
                        ps[:, h * 64:(h + 1) * 64])

            # ---- attention ----
            # q is processed in 1024-wide panels: two 512-wide scores matmuls
            # share one [128, 1024] PSUM tile so a single ACT exp covers both
            # (the mask bias is per-partition = per-key, constant across q).
            # The attnV matmuls run one k-chunk BEHIND the scores (software
            # pipeline) so the PE never stalls waiting for the exp that feeds
            # them -- a gap-free PE keeps the HAM clock at 2.4 GHz.
            o_head = []
            for h in range(HL):
                o_head.append(pp.tile([DH, Q], BF16, tag=f"oh{h}", name=f"o_head{h}"))
            first_panel = True
            for qp in range(Q // 1024):
                q0 = qp * 1024
                for h in range(HL):
                    tl, po = h // 2, (h % 2) * 64
                    oA = psO.tile([65, 512], F32, tag="oA", name="oA")
                    oB = psO.tile([65, 512], F32, tag="oB", name="oB")

                    def attnv(p, kt, h=h, oA=oA, oB=oB):
                        for hf, o_ps in enumerate((oA, oB)):
                            nc.tensor.matmul(
                                o_ps[:],
                                (v_sb[kt][:, h * 65:h * 65 + 65]),
                                (p[:, hf * 512:(hf + 1) * 512]),
                                start=(kt == 0), stop=(kt == KTC - 1))

                    lagq = []
                    for kt in range(KTC):
                        if first_panel:
                            vproj(kt)
                        s_ps = psS.tile([128, 1024], F32, tag="s", name="s_ps")
                        for hf in range(2):
                            nc.tensor.matmul(
                                s_ps[:, hf * 512:(hf + 1) * 512],
                                (k_t[tl][po:po + 64, kt * 128:(kt + 1) * 128]),
                                (q_t[tl][po:po + 64,
                                         q0 + hf * 512:q0 + (hf + 1) * 512]),
                                start=True, stop=True)
                        p_sb = cb.tile([128, 1024], BF16, tag="p", bufs=6,
                                       name="p_sb")
                        nc.scalar.activation(
                            p_sb[:], s_ps[:], EXP,
                            bias=mask_sb[:, kt:kt + 1], scale=1.0)
                        lagq.append((p_sb, kt))
                        if len(lagq) > 2:
                            attnv(*lagq.pop(0))
                    for item in lagq:
                        attnv(*item)
                    first_panel = False
                    # normalize: o[dh, q] /= denom[q] (denom is o_ps row 64):
                    # stage denom in SBUF, broadcast over 64 partitions via a
                    # C=1 matmul, 64-lane fast reciprocal, then scale.
                    for hf, o_ps in enumerate((oA, oB)):
                        dn = cb.tile([65, 512], F32R, tag="dn", bufs=2,
                                     name="dn")
                        nc.vector.tensor_copy(dn[64:65, :], o_ps[64:65, :])
                        bc_ps = psA.tile([64, 512], F32, tag="proj",
                                         name="bc_ps")
                        nc.tensor.matmul(bc_ps[:], (ones_sb[64:65, :]),
                                         (dn[64:65, :]), start=True, stop=True)
                        inv_sb = cb.tile([64, 512], F32, tag="invb", bufs=2,
                                         name="inv_sb")
                        nc.vector.reciprocal_approx_fast(inv_sb[:], bc_ps[:])
                        nc.vector.tensor_mul(
                            o_head[h][:, q0 + hf * 512:q0 + (hf + 1) * 512],
                            o_ps[0:64, :], inv_sb[:])

                # ---- output projection for this q-panel, on the psA slots
                # (idle during attention), overlapping the next panel ----
                for ot in range(4):
                    for qs in (2 * qp, 2 * qp + 1):
                        y_ps = psA.tile([128, 512], F32, tag="proj", name="ps")
                        for h in range(HL):
                            nc.tensor.matmul(
                                y_ps[:],
                                (wo[h][:, ot * 128:(ot + 1) * 128]),
                                (o_head[h][:, qs * 512:(qs + 1) * 512]),
                                start=(h == 0), stop=(h == HL - 1))
                        y_sb = cb.tile([128, 512], F32, tag="y", bufs=2,
                                       name="y_sb")
                        nc.vector.tensor_copy(y_sb[:], y_ps[:])
                        nc.sync.dma_start(
                            y_d[ot * 128:(ot + 1) * 128,
                                qs * 512:(qs + 1) * 512],
                            y_sb[:])

    nc.compile()
    return nc


def make_in_maps(queries, keys, values, valid_lens, W_q, W_k, W_v, W_o, KT):
    queries = np.asarray(queries, np.float32)
    keys = np.asarray(keys, np.float32)
    values = np.asarray(values, np.float32)
    W_q = np.asarray(W_q, np.float32)
    W_k = np.asarray(W_k, np.float32)
    W_v = np.asarray(W_v, np.float32)
    W_o = np.asarray(W_o, np.float32)
    vl = np.asarray(valid_lens).astype(np.int64)
    in_maps = []
    for c in range(N_CORES):
        b, hg = c // 2, c % 2
        sl = slice(hg * DL, (hg + 1) * DL)
        m = np.where(np.arange(KT) < vl[b], 0.0, NEG).astype(np.float32)
        bf = ml_dtypes.bfloat16
        in_maps.append({
            "xq_t": np.ascontiguousarray(queries[b].T).astype(bf),
            "xk_t": np.ascontiguousarray(keys[b, :KT].T).astype(bf),
            "xv_t": np.ascontiguousarray(values[b, :KT].T).astype(bf),
            "wq_t": np.ascontiguousarray((W_q[sl, :] / 8.0).T).astype(bf),
            "wk_t": np.ascontiguousarray(W_k[sl, :].T).astype(bf),
            "wv_t": np.ascontiguousarray(W_v[sl, :].T).astype(bf),
            "wo_t": np.ascontiguousarray(W_o[:, sl].T).astype(bf),
            "mask": np.ascontiguousarray(m.reshape(KT // 128, 128).T),
        })
    return in_maps


def pick_kt(valid_lens):
    vl_max = int(np.asarray(valid_lens).max())
    return int(min(KSEQ, max(128, ((vl_max + 127) // 128) * 128)))


def kernel(queries, keys, values, valid_lens, W_q, W_k, W_v, W_o):
    KT = pick_kt(valid_lens)
    nc = build_nc(KT)
    in_maps = make_in_maps(queries, keys, values, valid_lens,
                           W_q, W_k, W_v, W_o, KT)
    res = run_bass_kernel_spmd(nc, in_maps, list(range(N_CORES))).results
    out = np.empty((B, Q, D), np.float32)
    for b in range(B):
        out[b] = (res[2 * b]["y_t"] + res[2 * b + 1]["y_t"]).T
    return out


# revision 25
# speedup vs baseline: 1.2833x; 1.2833x over previous
"""Multi-head attention (B=4, Q=K=2048, D=512, H=8) on 8 TRN2 NeuronCores.

Sharding: data-parallel over batch across core pairs (4 batches x 2 cores),
tensor-parallel over heads within each pair (each core owns 4 of the 8 heads:
column-sharded W_q/W_k/W_v, row-sharded W_o).  Each core emits a partial
output projection for its batch; the host sums the two partials per batch.

Device-side layout choices:
  * All activations live transposed ([feature, seq]) so every matmul contracts
    over the partition dim with no on-chip transposes.
  * Scores are computed transposed (S_T[k, q] = K_h @ Q_h^T) so the valid-len
    key padding mask is a per-partition bias on the ACT exp instruction, and
    softmax needs no max-subtraction pass (scores are O(1) here; exp of the
    -1e6 masked entries underflows to exactly 0, matching the reference).
  * A ones-column interleaved into V makes the attnV matmul emit the softmax
    denominator for free (output row 64 of each head's [65, q] PSUM tile).
  * The key dim is truncated to max(valid_lens) rounded up to 128: dropped
    keys all have softmax weight exactly 0, so this is exact.
  * The whole matmul pipeline runs in bf16 with fp32 PSUM accumulation
    (plain fp32 matmuls are 4x slower on the PE and fp32 weight loads can't
    use fast-weight-load); softmax/normalization stay fp32.  Host converts
    inputs to bf16, which also halves the input DMA traffic.
"""

import ml_dtypes
import numpy as np

import concourse.bacc as bacc
import concourse.bass as bass
import concourse.mybir as mybir
from concourse import tile
from concourse.bass_utils import run_bass_kernel_spmd

F32 = mybir.dt.float32
F32R = mybir.dt.float32r
BF16 = mybir.dt.bfloat16

B, Q, KSEQ, D, H = 4, 2048, 2048, 512, 8
DH = D // H          # 64  head dim
HL = H // 2          # 4   local heads per core
DL = HL * DH         # 256 local features per core
NEG = -1.0e6
N_CORES = 8


def build_nc(KT: int):
    """Build the single-core SPMD program for a key length of KT (mult of 128)."""
    assert KT % 128 == 0 and 128 <= KT <= KSEQ
    KTC = KT // 128                      # number of 128-wide key chunks
    NQ = Q // 512                        # 4 q-chunks of 512
    KCH = [(s, min(512, KT - s)) for s in range(0, KT, 512)]
    EXP = mybir.ActivationFunctionType.Exp

    nc = bacc.Bacc("TRN2", target_bir_lowering=False, debug=False,
                   num_devices=N_CORES)

    def din(name, shape, dt=BF16):
        return nc.dram_tensor(name, shape, dt, kind="ExternalInput").ap()

    xq_d = din("xq_t", [D, Q])
    xk_d = din("xk_t", [D, KT])
    xv_d = din("xv_t", [D, KT])
    wq_d = din("wq_t", [D, DL])
    wk_d = din("wk_t", [D, DL])
    wv_d = din("wv_t", [D, DL])
    wo_d = din("wo_t", [DL, D])
    mask_d = din("mask", [128, KTC], F32)
    y_d = nc.dram_tensor("y_t", [D, Q], F32, kind="ExternalOutput").ap()

    with tile.TileContext(nc) as tc:
        with (
            # bf16 rounding on PSUM->SBUF copies is deliberate (see docstring)
            nc.allow_low_precision(reason="bf16 matmul operands"),
            tc.tile_pool(name="persist", bufs=1) as pp,
            tc.tile_pool(name="xpool", bufs=8) as xp,
            tc.tile_pool(name="cbuf", bufs=1) as cb,
            # 8 PSUM banks: psA 2x[128,512] (projections / broadcast / output
            # projection), psS 2x[128,1024] score tiles, psO 2x[65,512]
            # attention accumulators.
            tc.tile_pool(name="psA", bufs=2, space=bass.MemorySpace.PSUM) as psA,
            tc.tile_pool(name="psS", bufs=2, space=bass.MemorySpace.PSUM) as psS,
            tc.tile_pool(name="psO", bufs=1, space=bass.MemorySpace.PSUM) as psO,
        ):
            # ---- constants / weights / mask ----
            wq = [pp.tile([128, DL], BF16, tag=f"wq{i}", name=f"wq{i}") for i in range(4)]
            wk = [pp.tile([128, DL], BF16, tag=f"wk{i}", name=f"wk{i}") for i in range(4)]
            wv = [pp.tile([128, DL], BF16, tag=f"wv{i}", name=f"wv{i}") for i in range(4)]
            wo = [pp.tile([64, D], BF16, tag=f"wo{i}", name=f"wo{i}") for i in range(HL)]
            for i in range(4):
                nc.sync.dma_start(wq[i][:], wq_d[i * 128:(i + 1) * 128, :])
            mask_sb = pp.tile([128, KTC], F32, tag="mask", name="mask_sb")
            nc.sync.dma_start(mask_sb[:], mask_d[:])
            onescr = pp.tile([128, DH], F32, tag="onescr", name="onescr")
            nc.vector.memset(onescr[:], 1.0)
            # row 64 is the broadcast-matmul lhsT (must share base partition
            # with the denominator row it multiplies against)
            ones_sb = pp.tile([65, DH], F32R, tag="ones", name="ones_sb")
            nc.vector.tensor_copy(ones_sb[64:65, :], onescr[64:65, :])

            # ---- Q projection:  q_t[o, q] = (Wq_loc/8) @ x_q  (transposed) ----
            # first input is DMA'd in 512-column chunks so the first matmul
            # group can start as early as possible
            xq = [xp.tile([128, Q], BF16, tag="x", name=f"x{i}") for i in range(4)]
            for qs in range(NQ):
                for i in range(4):
                    nc.sync.dma_start(
                        xq[i][:, qs * 512:(qs + 1) * 512],
                        xq_d[i * 128:(i + 1) * 128, qs * 512:(qs + 1) * 512])
            for i in range(4):
                nc.sync.dma_start(wk[i][:], wk_d[i * 128:(i + 1) * 128, :])
                nc.sync.dma_start(wv[i][:], wv_d[i * 128:(i + 1) * 128, :])
            for i in range(HL):
                nc.sync.dma_start(wo[i][:], wo_d[i * 64:(i + 1) * 64, :])
            q_t = [pp.tile([128, Q], BF16, tag=f"q_t{i}", name=f"q_t{i}") for i in range(2)]

            def qproj(ot, qs):
                ps = psA.tile([128, 512], F32, tag="proj", name="ps")
                for ic in range(4):
                    nc.tensor.matmul(
                        ps[:],
                        (wq[ic][:, ot * 128:(ot + 1) * 128]),
                        (xq[ic][:, qs * 512:(qs + 1) * 512]),
                        start=(ic == 0), stop=(ic == 3))
                nc.vector.tensor_copy(q_t[ot][:, qs * 512:(qs + 1) * 512], ps[:])

            for qs in range(NQ):
                qproj(0, qs)

            # ---- K projection:  k_t[o, k] ----
            xk = [xp.tile([128, Q], BF16, tag="x", name=f"x{i}") for i in range(4)]
            for i in range(4):
                nc.sync.dma_start(xk[i][:, :KT], xk_d[i * 128:(i + 1) * 128, :])
            k_t = [pp.tile([128, KT], BF16, tag=f"k_t{i}", name=f"k_t{i}") for i in range(2)]

            def kproj(ot, s, w):
                ps = psA.tile([128, 512], F32, tag="proj", name="ps")
                for ic in range(4):
                    nc.tensor.matmul(
                        ps[:, :w],
                        (wk[ic][:, ot * 128:(ot + 1) * 128]),
                        (xk[ic][:, s:s + w]),
                        start=(ic == 0), stop=(ic == 3))
                nc.vector.tensor_copy(k_t[ot][:, s:s + w], ps[:, :w])

            for (s, w) in KCH:
                kproj(0, s, w)
            for qs in range(NQ):
                qproj(1, qs)
            for (s, w) in KCH:
                kproj(1, s, w)

            # ---- V projection (emitted lazily, interleaved into the first
            # attention panel so the PE stream has no separate V phase and the
            # ACT engine starts on exps ~35us earlier).  Natural layout
            # v[k, o], heads interleaved with a ones column: per key-chunk
            # tile [128, 4*65], col h*65+64 == 1 (gives the softmax
            # denominator for free in the attnV matmul).
            xv = [xp.tile([128, Q], BF16, tag="x", name=f"x{i}") for i in range(4)]
            for i in range(4):
                nc.sync.dma_start(xv[i][:, :KT], xv_d[i * 128:(i + 1) * 128, :])
            v_sb = [pp.tile([128, HL * 65], BF16, tag=f"v{kt}", name=f"v{kt}") for kt in range(KTC)]

            def vproj(kt):
                ps = psA.tile([128, 512], F32, tag="proj", name="ps")
                for ic in range(4):
                    nc.tensor.matmul(
                        ps[:, :DL],
                        (xv[ic][:, kt * 128:(kt + 1) * 128]),
                        (wv[ic][:]),
                        start=(ic == 0), stop=(ic == 3))
                nc.vector.tensor_copy(v_sb[kt][:, 64::65], onescr[:, 0:HL])
                for h in range(HL):
                    nc.vector.tensor_copy(
                        v_sb[kt][:, h * 65:h * 65 + 64],
                        ps[:, h * 64:(h + 1) * 64])

            # ---- attention ----
            # q is processed in 1024-wide panels: two 512-wide scores matmuls
            # share one [128, 1024] PSUM tile so a single ACT exp covers both
            # (the mask bias is per-partition = per-key, constant across q).
            # The attnV matmuls run one k-chunk BEHIND the scores (software
            # pipeline) so the PE never stalls waiting for the exp that feeds
            # them -- a gap-free PE keeps the HAM clock at 2.4 GHz.
            o_head = []
            for h in range(HL):
                o_head.append(pp.tile([DH, Q], BF16, tag=f"oh{h}", name=f"o_head{h}"))
            first_panel = True
            for qp in range(Q // 1024):
                q0 = qp * 1024
                for h in range(HL):
                    tl, po = h // 2, (h % 2) * 64
                    oA = psO.tile([65, 512], F32, tag="oA", name="oA")
                    oB = psO.tile([65, 512], F32, tag="oB", name="oB")

                    def attnv(p, kt, h=h, oA=oA, oB=oB):
                        for hf, o_ps in enumerate((oA, oB)):
                            nc.tensor.matmul(
                                o_ps[:],
                                (v_sb[kt][:, h * 65:h * 65 + 65]),
                                (p[:, hf * 512:(hf + 1) * 512]),
                                start=(kt == 0), stop=(kt == KTC - 1))

                    prev = None
                    for kt in range(KTC):
                        if first_panel:
                            vproj(kt)
                        s_ps = psS.tile([128, 1024], F32, tag="s", name="s_ps")
                        for hf in range(2):
                            nc.tensor.matmul(
                                s_ps[:, hf * 512:(hf + 1) * 512],
                                (k_t[tl][po:po + 64, kt * 128:(kt + 1) * 128]),
                                (q_t[tl][po:po + 64,
                                         q0 + hf * 512:q0 + (hf + 1) * 512]),
                                start=True, stop=True)
                        p_sb = cb.tile([128, 1024], BF16, tag="p", bufs=4,
                                       name="p_sb")
                        nc.scalar.activation(
                            p_sb[:], s_ps[:], EXP,
                            bias=mask_sb[:, kt:kt + 1], scale=1.0)
                        if prev is not None:
                            attnv(*prev)
                        prev = (p_sb, kt)
                    attnv(*prev)
                    first_panel = False
                    # normalize: o[dh, q] /= denom[q] (denom is o_ps row 64):
                    # stage denom in SBUF, broadcast over 64 partitions via a
                    # C=1 matmul, 64-lane fast reciprocal, then scale.
                    for hf, o_ps in enumerate((oA, oB)):
                        dn = cb.tile([65, 512], F32R, tag="dn", bufs=2,
                                     name="dn")
                        nc.vector.tensor_copy(dn[64:65, :], o_ps[64:65, :])
                        bc_ps = psA.tile([64, 512], F32, tag="proj",
                                         name="bc_ps")
                        nc.tensor.matmul(bc_ps[:], (ones_sb[64:65, :]),
                                         (dn[64:65, :]), start=True, stop=True)
                        inv_sb = cb.tile([64, 512], F32, tag="invb", bufs=2,
                                         name="inv_sb")
                        nc.vector.reciprocal_approx_fast(inv_sb[:], bc_ps[:])
                        nc.vector.tensor_mul(
                            o_head[h][:, q0 + hf * 512:q0 + (hf + 1) * 512],
                            o_ps[0:64, :], inv_sb[:])

                # ---- output projection for this q-panel, on the psA slots
                # (idle during attention), overlapping the next panel ----
                for ot in range(4):
                    for qs in (2 * qp, 2 * qp + 1):
                        y_ps = psA.tile([128, 512], F32, tag="proj", name="ps")
                        for h in range(HL):
                            nc.tensor.matmul(
                                y_ps[:],
                                (wo[h][:, ot * 128:(ot + 1) * 128]),
                                (o_head[h][:, qs * 512:(qs + 1) * 512]),
                                start=(h == 0), stop=(h == HL - 1))
                        y_sb = cb.tile([128, 512], F32, tag="y", bufs=2,
                                       name="y_sb")
                        nc.vector.tensor_copy(y_sb[:], y_ps[:])
                        nc.sync.dma_start(
                            y_d[ot * 128:(ot + 1) * 128,
                                qs * 512:(qs + 1) * 512],
                            y_sb[:])

    nc.compile()
    return nc


def make_in_maps(queries, keys, values, valid_lens, W_q, W_k, W_v, W_o, KT):
    queries = np.asarray(queries, np.float32)
    keys = np.asarray(keys, np.float32)
    values = np.asarray(values, np.float32)
    W_q = np.asarray(W_q, np.float32)
    W_k = np.asarray(W_k, np.float32)
    W_v = np.asarray(W_v, np.float32)
    W_o = np.asarray(W_o, np.float32)
    vl = np.asarray(valid_lens).astype(np.int64)
    in_maps = []
    for c in range(N_CORES):
        b, hg = c // 2, c % 2
        sl = slice(hg * DL, (hg + 1) * DL)
        m = np.where(np.arange(KT) < vl[b], 0.0, NEG).astype(np.float32)
        bf = ml_dtypes.bfloat16
        in_maps.append({
            "xq_t": np.ascontiguousarray(queries[b].T).astype(bf),
            "xk_t": np.ascontiguousarray(keys[b, :KT].T).astype(bf),
            "xv_t": np.ascontiguousarray(values[b, :KT].T).astype(bf),
            "wq_t": np.ascontiguousarray((W_q[sl, :] / 8.0).T).astype(bf),
            "wk_t": np.ascontiguousarray(W_k[sl, :].T).astype(bf),
            "wv_t": np.ascontiguousarray(W_v[sl, :].T).astype(bf),
            "wo_t": np.ascontiguousarray(W_o[:, sl].T).astype(bf),
            "mask": np.ascontiguousarray(m.reshape(KT // 128, 128).T),
        })
    return in_maps


def pick_kt(valid_lens):
    vl_max = int(np.asarray(valid_lens).max())
    return int(min(KSEQ, max(128, ((vl_max + 127) // 128) * 128)))


def kernel(queries, keys, values, valid_lens, W_q, W_k, W_v, W_o):
    KT = pick_kt(valid_lens)
    nc = build_nc(KT)
    in_maps = make_in_maps(queries, keys, values, valid_lens,
                           W_q, W_k, W_v, W_o, KT)
    res = run_bass_kernel_spmd(nc, in_maps, list(range(N_CORES))).results
    out = np.empty((B, Q, D), np.float32)
    for b in range(B):
        out[b] = (res[2 * b]["y_t"] + res[2 * b + 1]["y_t"]).T
    return out
